# revision 4
# baseline (speedup 1.0000x reference)
"""Multi-head attention (B=8, N=1024, D=1024, H=8) on 8 trn2 NeuronCores.

Sharding: data-parallel over batch — core i computes batch i entirely
(8 heads of QK^T -> softmax -> PV -> output projection). No collectives.

Per-core Bass/Tile program (all fp32, matmuls in float32r for full PE rate):
  - Q^T, K^T head slices built with PE transposes (fp32 has no DMA transpose).
  - S = Q K^T * scale via matmul(lhsT=Q^T block, rhs=K^T), PSUM [128,1024].
  - exp + row-sum fused in one scalar-engine activation (accum_out).
  - P = E * (1/rowsum) on DVE; DMA'd out as the attn output.
  - P blocks PE-transposed into P^T; PV: matmul(lhsT=V chunk, rhs=P^T)
    accumulated over j-chunks -> O^T [d, i] (already softmax-normalized).
  - Projection: matmul(lhsT=O^T block, rhs=W^T) accumulated over head
    chunks -> Y [i, m]; bias added from a partition-broadcast tile.
"""

import numpy as np

import concourse.bass as bass
import concourse.tile as tile
from concourse import bacc
from concourse import mybir
from concourse.masks import make_identity
from concourse.bass_utils import run_bass_kernel_spmd

B, N, D, H = 8, 1024, 1024, 8
HD = D // H          # 128, head dim = one partition tile
C = N // 128         # 8 chunks of 128 rows
SCALE = HD ** -0.5
F32 = mybir.dt.float32
F32R = mybir.dt.float32r
EXP = mybir.ActivationFunctionType.Exp

_PROGRAM = None


def _build_program():
    nc = bacc.Bacc()
    q = nc.declare_dram_parameter("q", [N, D], F32, isOutput=False)
    k = nc.declare_dram_parameter("k", [N, D], F32, isOutput=False)
    v = nc.declare_dram_parameter("v", [N, D], F32, isOutput=False)
    w = nc.declare_dram_parameter("w", [D, D], F32, isOutput=False)
    b = nc.declare_dram_parameter("b", [D], F32, isOutput=False)
    out = nc.declare_dram_parameter("out", [N, D], F32, isOutput=True)
    attn = nc.declare_dram_parameter("attn", [H, N, N], F32, isOutput=True)

    with tile.TileContext(nc) as tc:
        with (
            tc.tile_pool(name="const", bufs=1) as const,
            tc.tile_pool(name="wrow", bufs=2) as wrow_pool,
            tc.tile_pool(name="qkblk", bufs=4) as qkblk,
            tc.tile_pool(name="qt", bufs=2) as qt_pool,
            tc.tile_pool(name="vt", bufs=2) as vt_pool,
            tc.tile_pool(name="pt", bufs=1) as pt_pool,
            tc.tile_pool(name="ep", bufs=2) as e_pool,
            tc.tile_pool(name="pp", bufs=3) as p_pool,
            tc.tile_pool(name="yp", bufs=2) as y_pool,
            tc.tile_pool(name="rp", bufs=4) as r_pool,
            tc.tile_pool(name="spsum", bufs=2, space="PSUM") as spsum,
            tc.tile_pool(name="apsum", bufs=1, space="PSUM") as apsum,
            tc.tile_pool(name="tpsum", bufs=2, space="PSUM") as tpsum,
        ):
            ident = const.tile([128, 128], F32)
            make_identity(nc, ident)

            # W^T resident in SBUF: wt[:, c, m] = W[m, c*128 + :]
            wt = const.tile([128, C, D], F32R)
            for mc in range(C):
                wrow = wrow_pool.tile([128, D], F32)
                nc.sync.dma_start(wrow, w[mc * 128:(mc + 1) * 128, :])
                for c in range(C):
                    ps = tpsum.tile([128, 128], F32, tag="tp")
                    nc.tensor.transpose(ps, wrow[:, c * 128:(c + 1) * 128], ident)
                    nc.any.tensor_copy(wt[:, c, mc * 128:(mc + 1) * 128], ps)

            # bias broadcast to all partitions
            b_row = const.tile([1, D], F32)
            nc.sync.dma_start(b_row, b.rearrange("(o d) -> o d", o=1))
            bb = const.tile([128, D], F32)
            nc.gpsimd.partition_broadcast(bb, b_row)

            # O^T accumulator across heads: ot[:, h, i] = O_h^T[d, i]
            ot = const.tile([128, H, D], F32R)

            for h in range(H):
                hs = slice(h * HD, (h + 1) * HD)
                qt = qt_pool.tile([128, N], F32R, tag="qt")
                kt = qt_pool.tile([128, N], F32R, tag="kt")
                for c in range(C):
                    rows = slice(c * 128, (c + 1) * 128)
                    qb = qkblk.tile([128, 128], F32, tag="qb")
                    nc.sync.dma_start(qb, q[rows, hs])
                    ps = tpsum.tile([128, 128], F32, tag="tp")
                    nc.tensor.transpose(ps, qb, ident)
                    nc.any.tensor_copy(qt[:, rows], ps)
                    kb = qkblk.tile([128, 128], F32, tag="kb")
                    nc.sync.dma_start(kb, k[rows, hs])
                    ps2 = tpsum.tile([128, 128], F32, tag="tp")
                    nc.tensor.transpose(ps2, kb, ident)
                    nc.any.tensor_copy(kt[:, rows], ps2)
                vt_raw = vt_pool.tile([128, C, HD], F32, tag="vt_raw")
                nc.sync.dma_start(
                    vt_raw, v[:, hs].rearrange("(jc ji) d -> ji jc d", ji=128)
                )
                vt = vt_pool.tile([128, C, HD], F32R, tag="vt")
                nc.any.tensor_copy(vt, vt_raw)

                pt = pt_pool.tile([128, C, N], F32R)
                for ic in range(C):
                    irows = slice(ic * 128, (ic + 1) * 128)
                    sp = spsum.tile([128, N], F32, tag="s")
                    qt_blk = qt[:, irows]
                    nc.tensor.matmul(sp[:, 0:512], lhsT=qt_blk,
                                     rhs=kt[:, 0:512],
                                     start=True, stop=True)
                    nc.tensor.matmul(sp[:, 512:1024], lhsT=qt_blk,
                                     rhs=kt[:, 512:1024],
                                     start=True, stop=True)
                    e = e_pool.tile([128, N], F32)
                    r = r_pool.tile([128, 1], F32, tag="r")
                    nc.scalar.activation(e, sp, EXP, scale=SCALE, accum_out=r)
                    rc = r_pool.tile([128, 1], F32, tag="rc")
                    nc.vector.reciprocal(rc, r)
                    p = p_pool.tile([128, N], F32)
                    nc.vector.tensor_scalar_mul(p, e, rc)
                    nc.sync.dma_start(attn[h, irows, :], p)
                    for jb in range(C):
                        tps = tpsum.tile([128, 128], F32, tag="tp")
                        nc.tensor.transpose(tps, p[:, jb * 128:(jb + 1) * 128], ident)
                        nc.any.tensor_copy(pt[:, jb, irows], tps)

                op = apsum.tile([128, N], F32, tag="acc")
                for jc in range(C):
                    vt_blk = vt[:, jc, :]
                    nc.tensor.matmul(op[:, 0:512], lhsT=vt_blk,
                                     rhs=pt[:, jc, 0:512],
                                     start=(jc == 0), stop=(jc == C - 1))
                for jc in range(C):
                    vt_blk = vt[:, jc, :]
                    nc.tensor.matmul(op[:, 512:1024], lhsT=vt_blk,
                                     rhs=pt[:, jc, 512:1024],
                                     start=(jc == 0), stop=(jc == C - 1))
                nc.any.tensor_copy(ot[:, h, :], op)

            # output projection: Y[i, m] = sum_h O_h^T[:, i].T @ W^T[h] + b
            for ic in range(C):
                irows = slice(ic * 128, (ic + 1) * 128)
                yp = apsum.tile([128, N], F32, tag="acc")
                for hc in range(H):
                    o_blk = ot[:, hc, irows]
                    nc.tensor.matmul(yp[:, 0:512], lhsT=o_blk,
                                     rhs=wt[:, hc, 0:512],
                                     start=(hc == 0), stop=(hc == H - 1))
                for hc in range(H):
                    o_blk = ot[:, hc, irows]
                    nc.tensor.matmul(yp[:, 512:1024], lhsT=o_blk,
                                     rhs=wt[:, hc, 512:1024],
                                     start=(hc == 0), stop=(hc == H - 1))
                y = y_pool.tile([128, N], F32)
                nc.vector.tensor_tensor(y, yp, bb, mybir.AluOpType.add)
                nc.sync.dma_start(out[irows, :], y)

    nc.compile()
    return nc


def _get_program():
    global _PROGRAM
    if _PROGRAM is None:
        _PROGRAM = _build_program()
    return _PROGRAM


def kernel(q, k, v, W_out, b_out, **run_kwargs):
    q = np.ascontiguousarray(np.asarray(q, dtype=np.float32))
    k = np.ascontiguousarray(np.asarray(k, dtype=np.float32))
    v = np.ascontiguousarray(np.asarray(v, dtype=np.float32))
    W_out = np.ascontiguousarray(np.asarray(W_out, dtype=np.float32))
    b_out = np.ascontiguousarray(np.asarray(b_out, dtype=np.float32))

    nc = _get_program()
    in_maps = [
        {"q": q[i], "k": k[i], "v": v[i], "w": W_out, "b": b_out}
        for i in range(B)
    ]
    res = run_bass_kernel_spmd(nc, in_maps, list(range(B)), **run_kwargs)
    out = np.stack([res.results[i]["out"] for i in range(B)])
    attn = np.stack([res.results[i]["attn"] for i in range(B)])
    if run_kwargs.get("trace"):
        return (out, attn), res
    return out, attn
